# revision 12
# baseline (speedup 1.0000x reference)
"""CavemanGPT single-head attention on 8 Trainium2 NeuronCores.

Math (reference; its mask input is unused there):
    Q = emb @ W_q^T ; K = emb @ W_k^T ; V = emb @ W_v^T        (per batch b)
    out = softmax(K @ Q^T / sqrt(H), axis=-1) @ V

Algebraic restructure: K @ Q^T = emb @ (W_k^T W_q) @ emb^T, so with
G := W_k^T @ W_q  ([E, E], batch independent) the per-core work drops from
~52 GFLOP to ~16 GFLOP and the giant [S, H] Q/K intermediates vanish:
    AT := G^T @ emb_i^T            ([E, SI])
    scores = AT^T @ emb^T          ([SI, S])
    out = softmax(...) @ V

Precision strategy (replaces the old 3x-fp16-limb scheme): the PE's
float32r mode reads fp32 operands, truncates to e8m13 (14-bit mantissa),
and streams at fp16 speed for moving dims >= 256 (HW-measured 230 ns vs
217 ns per [128x128]x[128x512] matmul). 14 bits is NOT enough for the raw
chain: G ~= 1024*J + noise, so scores (~1e7) are dominated by a rank-2
row/column structure and the softmax needs ~raw-unit accuracy near the max.
Fix: two-way center G on the host between launches,
    G = G~ + r 1^T + 1 c^T - mu J,   G~ std ~5.7 (vs 1024),
run the AT/scores chain on G~ in fp32r (errors now ~2 raw units), and add
the exact rank-2 correction
    scores_ij = scores~_ij + u_i p_j + p_i w_j
        p = emb @ 1, u = emb @ r, w = emb @ c - mu p   (host fp64)
on-device via DVE in true fp32 during PSUM evacuation. V and attn@V are
post-softmax (error passes through linearly) and stay single fp16.
Numpy-simulated end-to-end rel err: 7.6e-3 (gate: 2e-2).

Two launches:
  1. G-launch: G = W_k^T @ W_q sharded over 8 cores (2 e'-halves x 4
     h-quarters) in one fp32r pass; host sums the h-partials (fp64). Each
     core also emits one (batch, j-half) shard of V = emb @ W_v^T (fp16).
  2. Main launch: 8 cores = 4 batches x 2 halves of the i (output-row)
     dimension. Each core receives its batch's emb^T (fp32) with its own
     i-half's columns permuted to the front (softmax over j is permutation
     invariant) and produces out[i-half].
"""

import math

import numpy as np

import concourse.bass as bass
import concourse.bass_utils as _bu
import concourse.mybir as mybir
import concourse.tile as tile
from concourse import bacc
from concourse.bass_utils import run_bass_kernel_spmd
from concourse.masks import make_identity

dt = mybir.dt
P = 128
N_CORES = 8


def _rne12(x):
    """Round fp32 to 12 mantissa bits, nearest-even. The PE's fp32r mode
    truncates operands to 12 mantissa bits (HW-calibrated); feeding values
    already at 12 bits makes that truncation a no-op, so the effective
    rounding becomes this (unbiased, half-ulp) one."""
    x = np.ascontiguousarray(x, dtype=np.float32)
    drop = 11
    mask = np.uint64(0xFFFFF800)
    xi = x.view(np.uint32).astype(np.uint64)
    bias = ((xi >> np.uint64(drop)) & np.uint64(1)) + np.uint64((1 << (drop - 1)) - 1)
    return ((xi + bias) & mask).astype(np.uint32).view(np.float32)


def build_g_nc(S, E, H, O):
    """Launch 1: per-core partial G' = W_k[hq]^T @ W_q[hq][:, e'half] (fp32r)
    plus one (batch, j-half) shard of V = embT^T @ WvT (fp16).

    Core c handles G e'-half (c % 2) / h-quarter (c // 2), and V for batch
    (c // 2), j-half (c % 2). Host sums the G h-partials and reassembles V.
    """
    SI = S // 2
    EH = E // 2
    HQ = H // 4
    EB = E // P
    HCB = HQ // P
    JBH = SI // P
    OW = min(512, O)
    NOW = O // OW
    f32, f32r, f16 = dt.float32, dt.float32r, dt.float16

    nc = bacc.Bacc("TRN2", target_bir_lowering=False, debug=False)
    wk = nc.dram_tensor("wk", [HQ, E], f32r, kind="ExternalInput").ap()
    wq = nc.dram_tensor("wq", [HQ, EH], f32r, kind="ExternalInput").ap()
    evt = nc.dram_tensor("evt", [E, SI], f16, kind="ExternalInput").ap()
    wvt = nc.dram_tensor("wvt", [E, O], f16, kind="ExternalInput").ap()
    g_part = nc.dram_tensor("g_part", [E, EH], f32, kind="ExternalOutput").ap()
    v_part = nc.dram_tensor("v_part", [SI, O], f16, kind="ExternalOutput").ap()

    with tile.TileContext(nc) as tc:
        with (
            tc.tile_pool(name="p_res", bufs=1) as p_res,
            tc.tile_pool(name="p_vo", bufs=2) as p_vo,
            tc.tile_pool(name="p_gs", bufs=3) as p_gs,
            tc.tile_pool(name="ps_g", bufs=8, space="PSUM") as ps_g,
        ):
            # ---- PE warm-up: ~3.5us of dummy matmuls during the DMA
            # preamble trips the HAM clock-gate so real matmuls start at
            # 2.4GHz instead of 1.2 ----
            wu = p_res.tile([P, P], f16)
            nc.gpsimd.memset(wu[:], 0.0)
            wups = ps_g.tile([P, P], f32, tag="gps", name="wups")
            for _ in range(32):
                nc.tensor.matmul(wups[:], wu[:], wu[:], start=True, stop=True)

            # ---- G partial: single fp32r pass over host-centered W-hat ----
            gp = p_res.tile([P, EB, EH], f32)
            evc = p_res.tile([P, EB, SI], f16)
            wvc = p_res.tile([P, EB, O], f16)
            evtr = evt.rearrange("(eo p) j -> p eo j", p=P)
            wvtr = wvt.rearrange("(eo p) o -> p eo o", p=P)
            pt_g = [
                ps_g.tile([P, EH], f32, tag="gps", name=f"gps_{eb}")
                for eb in range(EB)
            ]
            for hc in range(HCB):
                hs = slice(hc * P, (hc + 1) * P)
                kc = p_gs.tile([P, E], f32r, tag="kc")
                nc.sync.dma_start(kc[:], wk[hs, :])
                qc = p_gs.tile([P, EH], f32r, tag="qc")
                nc.sync.dma_start(qc[:], wq[hs, :])
                first, last = hc == 0, hc == HCB - 1
                for eb in range(EB):
                    ksl = slice(eb * P, (eb + 1) * P)
                    nc.tensor.matmul(
                        pt_g[eb][:], kc[:, ksl], qc[:], start=first, stop=last
                    )
            # V inputs: per-eb chunks so V matmuls can start before the full
            # 4MB lands (V accumulates over eb; first group needs only eb=0)
            for eb in range(EB):
                nc.sync.dma_start(evc[:, eb], evtr[:, eb])
                nc.sync.dma_start(wvc[:, eb], wvtr[:, eb])
            gpr = g_part.rearrange("(eo p) e2 -> p eo e2", p=P)
            for eb in range(EB):
                nc.vector.tensor_copy(gp[:, eb], pt_g[eb][:])
                nc.sync.dma_start(gpr[:, eb], gp[:, eb])

            # ---- V shard: jb-groups of 4 so psum use stays within 8 banks
            # while the eb (DMA-chunk) loop is outermost ----
            for jg in range(0, JBH, 4):
                pv_tiles = [
                    [
                        ps_g.tile([P, OW], f32, tag="gps", name=f"vps_{jb}_{ob}")
                        for ob in range(NOW)
                    ]
                    for jb in range(jg, jg + 4)
                ]
                for eb in range(EB):
                    for ji, jb in enumerate(range(jg, jg + 4)):
                        jsl = slice(jb * P, (jb + 1) * P)
                        for ob in range(NOW):
                            osl = slice(ob * OW, (ob + 1) * OW)
                            nc.tensor.matmul(
                                pv_tiles[ji][ob][:], evc[:, eb, jsl],
                                wvc[:, eb, osl],
                                start=(eb == 0), stop=(eb == EB - 1),
                            )
                for ji, jb in enumerate(range(jg, jg + 4)):
                    jsl = slice(jb * P, (jb + 1) * P)
                    vt = p_vo.tile([P, O], f16, tag="vt")
                    for ob in range(NOW):
                        osl = slice(ob * OW, (ob + 1) * OW)
                        nc.vector.tensor_copy(vt[:, osl], pv_tiles[ji][ob][:])
                        nc.sync.dma_start(v_part[jsl, osl], vt[:, osl])

    nc.compile()
    return nc


def build_main_nc(S, E, H, O):
    """Launch 2: attention for one (batch, i-half); centered G~ as fp32."""
    SI = S // 2          # i rows per core
    EB = E // P          # 128-chunks of the embedding dim
    JB = S // P
    IB = SI // P
    IW = min(512, SI)    # AT moving width along i
    NIH = SI // IW
    JW = min(512, S)     # scores moving width along j
    NJW = S // JW
    OW = min(512, O)
    NOW = O // OW
    SCALE_EXP = 1.0 / math.sqrt(H)

    f32, f32r, f16 = dt.float32, dt.float32r, dt.float16

    nc = bacc.Bacc("TRN2", target_bir_lowering=False, debug=False)
    g_t = nc.dram_tensor("g_t", [E, E], f32r, kind="ExternalInput").ap()
    et_in = nc.dram_tensor("et_in", [E, S], f32r, kind="ExternalInput").ap()
    v_in = nc.dram_tensor("v_in", [S, O], f16, kind="ExternalInput").ap()
    bpw_in = nc.dram_tensor("bpw_in", [P, 2, S], f32, kind="ExternalInput").ap()
    ucp_in = nc.dram_tensor("ucp_in", [P, 2, IB], f32, kind="ExternalInput").ap()
    out = nc.dram_tensor("out", [SI, O], f32, kind="ExternalOutput").ap()

    with tile.TileContext(nc) as tc:
        with (
            tc.tile_pool(name="misc", bufs=2) as misc,
            tc.tile_pool(name="p_big", bufs=1) as p_big,
        ):
            ident = misc.tile([P, P], f16, tag="ident", name="ident")
            make_identity(nc, ident[:])
            wu = misc.tile([P, P], f16, tag="wu", name="wu")
            nc.gpsimd.memset(wu[:], 0.0)

            # whole-kernel residents
            et = p_big.tile([P, EB, S], f32r)    # embT: [e part, e chunk, tok]
            at = p_big.tile([P, EB, SI], f32r)   # AT~: [e' part, e' chunk, i]
            v16 = p_big.tile([P, JB, O], f16)    # V: [j part, j chunk, o]
            bpw = p_big.tile([P, 2, S], f32)     # broadcast rows: p_j, w_j
            ucp = p_big.tile([P, 2, IB], f32)    # per-i cols: u_i, p_i

            with tc.tile_pool(name="ps", bufs=8, space="PSUM") as ps:
                # PE warm-up during the input-DMA preamble (see launch 1)
                wups = ps.tile([P, P], f32, tag="ps", name="wups")
                for _ in range(32):
                    nc.tensor.matmul(wups[:], wu[:], wu[:], start=True, stop=True)

                # ---- AT~ = G~^T embT (fp32r) ----
                with tc.tile_pool(name="p_g", bufs=1) as p_g:
                    gt = p_g.tile([P, EB, E], f32r)  # [e part, e chunk, e']
                    # DMAs emitted in first-use order, chunked per e-block so
                    # the first AT matmuls start early.
                    gtr = g_t.rearrange("(eo p) e2 -> p eo e2", p=P)
                    etr = et_in.rearrange("(eo p) t -> p eo t", p=P)
                    for eb in range(EB):
                        nc.sync.dma_start(gt[:, eb], gtr[:, eb])
                        nc.sync.dma_start(et[:, eb, :SI], etr[:, eb, :SI])
                    if SI < S:
                        nc.sync.dma_start(et[:, :, SI:], etr[:, :, SI:])
                    nc.sync.dma_start(
                        v16[:], v_in.rearrange("(jo p) o -> p jo o", p=P)
                    )
                    nc.sync.dma_start(bpw[:], bpw_in)
                    nc.sync.dma_start(ucp[:], ucp_in)
                    # PSUM evac applies a Veltkamp split so `at` lands as its
                    # nearest 12-bit-mantissa value: the PE's fp32r RTZ-12
                    # truncation of it in the scores stage is then a no-op
                    # (unbiased, half the error of raw truncation).
                    VC = float(2**11 + 1)
                    for ih in range(NIH):
                        isl = slice(ih * IW, (ih + 1) * IW)
                        pts = [
                            ps.tile([P, IW], f32, tag="ps", name=f"aps_{ih}_{epb}")
                            for epb in range(EB)
                        ]
                        for eb in range(EB):
                            first, last = eb == 0, eb == EB - 1
                            for epb in range(EB):
                                psl = slice(epb * P, (epb + 1) * P)
                                nc.tensor.matmul(
                                    pts[epb][:], gt[:, eb, psl], et[:, eb, isl],
                                    start=first, stop=last,
                                )
                        for epb in range(EB):
                            vk1 = p_g.tile([P, IW], f32, tag="vk1",
                                           name=f"vk1_{ih}_{epb}")
                            nc.vector.tensor_scalar_mul(vk1[:], pts[epb][:], VC)
                            vk2 = p_g.tile([P, IW], f32, tag="vk2",
                                           name=f"vk2_{ih}_{epb}")
                            nc.vector.tensor_tensor(
                                vk2[:], vk1[:], pts[epb][:],
                                mybir.AluOpType.subtract,
                            )
                            nc.vector.tensor_tensor(
                                at[:, epb, isl], vk1[:], vk2[:],
                                mybir.AluOpType.subtract,
                            )

                # ---- scores + rank-2 corr + softmax + out, per i-block ----
                with (
                    tc.tile_pool(name="p_sc", bufs=1) as p_sc,
                    tc.tile_pool(name="p_sw", bufs=2) as p_sw,
                    tc.tile_pool(name="p_sw1", bufs=2) as p_sw1,
                    tc.tile_pool(name="p_corr", bufs=1) as p_corr,
                ):
                    def emit_scores(ib):
                        ibs = slice(ib * P, (ib + 1) * P)
                        pt_s = [
                            ps.tile([P, JW], f32, tag="ps", name=f"sps_{ib}_{w}")
                            for w in range(NJW)
                        ]
                        for epb in range(EB):
                            for w in range(NJW):
                                wsl = slice(w * JW, (w + 1) * JW)
                                nc.tensor.matmul(
                                    pt_s[w][:], at[:, epb, ibs], et[:, epb, wsl],
                                    start=(epb == 0), stop=(epb == EB - 1),
                                )
                        return pt_s

                    pt_s = emit_scores(0)
                    for ib in range(IB):
                        ibs = slice(ib * P, (ib + 1) * P)
                        # rank-2 correction: t12 = u_i*p_j + p_i*w_j
                        t1 = p_corr.tile([P, S], f32, tag="t1", name=f"t1_{ib}")
                        nc.vector.tensor_scalar_mul(
                            t1[:], bpw[:, 0], ucp[:, 0, ib : ib + 1]
                        )
                        t2 = p_corr.tile([P, S], f32, tag="t2", name=f"t2_{ib}")
                        nc.vector.tensor_scalar_mul(
                            t2[:], bpw[:, 1], ucp[:, 1, ib : ib + 1]
                        )
                        t12 = p_corr.tile([P, S], f32, tag="t12", name=f"t12_{ib}")
                        nc.vector.tensor_tensor(
                            t12[:], t1[:], t2[:], mybir.AluOpType.add
                        )
                        # evacuate PSUM + corr into fp32 scores tile (frees
                        # the PSUM banks for the next block's matmuls)
                        sc = p_sc.tile([P, S], f32, tag="sc", name=f"sc_{ib}")
                        for w in range(NJW):
                            wsl = slice(w * JW, (w + 1) * JW)
                            nc.vector.tensor_tensor(
                                sc[:, wsl], pt_s[w][:], t12[:, wsl],
                                mybir.AluOpType.add,
                            )
                        nmx = p_sw.tile([P, 1], f32, tag="nmx")
                        nc.vector.reduce_max(
                            nmx[:], sc[:], axis=mybir.AxisListType.X, negate=True
                        )
                        nmx2 = p_sw.tile([P, 1], f32, tag="nmx2")
                        nc.vector.tensor_scalar_mul(nmx2[:], nmx[:], SCALE_EXP)
                        # unnormalized exp, fp16; normalization is deferred to
                        # the output evacuation (x 1/sum per i-row)
                        attn16 = p_sw1.tile([P, S], f16, tag="attn16")
                        nc.scalar.activation(
                            attn16[:], sc[:],
                            mybir.ActivationFunctionType.Exp,
                            bias=nmx2[:], scale=SCALE_EXP,
                        )
                        sm = p_sw.tile([P, 1], f32, tag="sm")
                        nc.vector.reduce_sum(sm[:], attn16[:], axis=mybir.AxisListType.X)
                        rs = p_sw.tile([P, 1], f32, tag="rs")
                        nc.vector.reciprocal(rs[:], sm[:])
                        if ib + 1 < IB:
                            pt_s = emit_scores(ib + 1)
                        attnT = p_sw1.tile([P, JB, P], f16, tag="attnT")
                        for jb in range(JB):
                            tp = ps.tile([P, P], f16, tag="ps", name=f"tps_{ib}_{jb}")
                            nc.tensor.transpose(
                                tp[:], attn16[:, jb * P : (jb + 1) * P], ident[:]
                            )
                            nc.vector.tensor_copy(attnT[:, jb, :], tp[:])
                        pt_o = [
                            ps.tile([P, OW], f32, tag="ps", name=f"ops_{ib}_{ob}")
                            for ob in range(NOW)
                        ]
                        for jb in range(JB):
                            for ob in range(NOW):
                                nc.tensor.matmul(
                                    pt_o[ob][:],
                                    attnT[:, jb, :],
                                    v16[:, jb, ob * OW : (ob + 1) * OW],
                                    start=(jb == 0), stop=(jb == JB - 1),
                                )
                        outt = p_sw1.tile([P, O], f32, tag="outt")
                        for ob in range(NOW):
                            osl = slice(ob * OW, (ob + 1) * OW)
                            nc.vector.tensor_scalar_mul(
                                outt[:, osl], pt_o[ob][:], rs[:]
                            )
                            nc.sync.dma_start(out[ibs, osl], outt[:, osl])

    nc.compile()
    return nc


_NC_CACHE = {}


def _get_nc(builder, *key):
    k = (builder.__name__,) + key
    if k not in _NC_CACHE:
        _NC_CACHE[k] = builder(*key)
    return _NC_CACHE[k]


def kernel(token_emb, W_q, W_k, W_v, mask=None, _trace=False, _tmpdir=None):
    token_emb = np.asarray(token_emb, np.float32)
    W_q = np.asarray(W_q, np.float32)
    W_k = np.asarray(W_k, np.float32)
    W_v = np.asarray(W_v, np.float32)
    B, S, E = token_emb.shape
    H = W_q.shape[0]
    O = W_v.shape[0]
    SI = S // 2
    EH = E // 2
    HQ = H // 4
    IB = SI // P
    assert 2 * B == N_CORES

    # ---- launch 1: sharded G-hat = Wk-hat^T @ Wq-hat (fp32r, host-centered
    # W: W = 0.5 + W-hat halves the rounded magnitudes) and V = emb @ W_v^T ----
    nc_g = _get_nc(build_g_nc, S, E, H, O)
    wk_c = _rne12(W_k - np.float32(0.5))
    wq_c = _rne12(W_q - np.float32(0.5))
    wvt = np.ascontiguousarray(W_v.T).astype(np.float16)
    embT = [np.ascontiguousarray(token_emb[b].T) for b in range(B)]  # [E,S] f32
    embT16 = [e.astype(np.float16) for e in embT]
    g_maps = []
    for c in range(N_CORES):
        half, hq = c % 2, c // 2
        hsl = slice(hq * HQ, (hq + 1) * HQ)
        esl = slice(half * EH, (half + 1) * EH)
        b, jhalf = c // 2, c % 2
        g_maps.append(
            {
                "wk": np.ascontiguousarray(wk_c[hsl]),
                "wq": np.ascontiguousarray(wq_c[hsl, esl]),
                "evt": np.ascontiguousarray(
                    embT16[b][:, jhalf * SI : (jhalf + 1) * SI]
                ),
                "wvt": wvt,
            }
        )
    res_g = run_bass_kernel_spmd(
        nc_g, g_maps, core_ids=list(range(N_CORES)), trace=_trace,
        tmpdir=(_tmpdir + "/g" if _tmpdir else None),
    )
    # G = G-hat + exact rank-1 terms from the 0.5 offsets (host fp64)
    G = np.empty((E, E), np.float64)
    for half in range(2):
        esl = slice(half * EH, (half + 1) * EH)
        G[:, esl] = sum(
            res_g.results[2 * q + half]["g_part"].astype(np.float64)
            for q in range(4)
        )
    sk = (W_k.astype(np.float64) - 0.5).sum(axis=0)
    sq = (W_q.astype(np.float64) - 0.5).sum(axis=0)
    G += 0.5 * sk[:, None] + 0.5 * sq[None, :] + 0.25 * H
    v_nat = [
        np.concatenate(
            [res_g.results[2 * b + 0]["v_part"], res_g.results[2 * b + 1]["v_part"]],
            axis=0,
        )
        for b in range(B)
    ]

    # ---- host: two-way center G, build rank-2 correction vectors (fp64) ----
    r = G.mean(axis=1)
    cm = G.mean(axis=0)
    mu = G.mean()
    Gt = _rne12((G - r[:, None] - cm[None, :] + mu).astype(np.float32))

    # ---- launch 2: attention ----
    nc_main = _get_nc(build_main_nc, S, E, H, O)
    in_maps = []
    for c in range(N_CORES):
        b, half = divmod(c, 2)
        e64 = token_emb[b].astype(np.float64)
        p = e64.sum(axis=1)
        u = e64 @ r
        w = e64 @ cm - mu * p
        perm = np.concatenate(
            [np.arange(half * SI, (half + 1) * SI),
             np.arange((1 - half) * SI, (2 - half) * SI)]
        )
        et_p = _rne12(np.concatenate(
            [embT[b][:, half * SI : (half + 1) * SI],
             embT[b][:, (1 - half) * SI : (2 - half) * SI]],
            axis=1,
        ))
        p_p = p[perm].astype(np.float32)
        w_p = w[perm].astype(np.float32)
        u_p = u[perm].astype(np.float32)
        bpw = np.empty((P, 2, S), np.float32)
        bpw[:, 0, :] = p_p
        bpw[:, 1, :] = w_p
        ucp = np.empty((P, 2, IB), np.float32)
        ucp[:, 0, :] = u_p[:SI].reshape(IB, P).T
        ucp[:, 1, :] = p_p[:SI].reshape(IB, P).T
        vp = v_nat[b]
        v_in = np.concatenate(
            [vp[half * SI : (half + 1) * SI], vp[(1 - half) * SI : (2 - half) * SI]],
            axis=0,
        )
        in_maps.append(
            {
                "g_t": Gt, "et_in": np.ascontiguousarray(et_p),
                "v_in": np.ascontiguousarray(v_in),
                "bpw_in": bpw, "ucp_in": ucp,
            }
        )
    res = run_bass_kernel_spmd(
        nc_main, in_maps, core_ids=list(range(N_CORES)), trace=_trace,
        tmpdir=(_tmpdir + "/main" if _tmpdir else None),
    )

    out = np.empty((B, S, O), np.float32)
    for c in range(N_CORES):
        b, half = divmod(c, 2)
        out[b, half * SI : (half + 1) * SI] = res.results[c]["out"]
    if _trace:
        kernel._last_results = (res_g, res)
    return out
